# revision 1
# baseline (speedup 1.0000x reference)
"""Trainium2 Bass kernel for nn_ContrastiveDistortion (symmetric pairwise-KL InfoNCE loss).

Math: with IS_SYMMETRIC=True the logdet terms cancel and
  logits_sym[a,b] = D/2 - U[a,b]/4,  U = tr(a,b)+tr(b,a)+quad(a,b)+quad(b,a)
U is a single inner product of stacked feature planes (K=6*128) plus a rank-1
column term c_b (row term c_a cancels in log-softmax). Each of the 8 cores gets
the full [128,4096] feature-major mu/sigma, column-ROTATED by 512*k so that the
program is SPMD-identical: the core's own 512-row block is always local columns
0..511 (diagonal masked there) and the positive pairs are local columns
2048..2559. Row-block softmax uses two per-half online logsumexps combined at
the end; per-core partial row-loss sums are reduced on host.
"""

import sys
from contextlib import ExitStack

import numpy as np

sys.path.insert(0, "/opt/trn_rl_repo")

import concourse.bass as bass
import concourse.bacc as bacc_mod
import concourse.mybir as mybir
from concourse.bass_utils import run_bass_kernel_spmd
from concourse.tile import TileContext

F32 = mybir.dt.float32
F32R = mybir.dt.float32r
I32 = mybir.dt.int32
AF = mybir.ActivationFunctionType
ALU = mybir.AluOpType
AX = mybir.AxisListType

P = 128          # partitions / feature dim D
NB = 4096        # N = 2B rows
NC = 8           # cores
RB = NB // NC    # 512 rows per core
NM = RB // P     # 4 m-chunks of 128 rows
HALF = NB // 2   # 2048 columns per softmax half
TEMPERATURE = 0.1
WEIGHT = 5.0
SCL = 1.0 / (4.0 * TEMPERATURE)  # 2.5: l = -SCL*U + const_row


def _build_nc():
    nc = bacc_mod.Bacc(None, target_bir_lowering=False, name="contrastive_distortion")
    muT_d = nc.declare_dram_parameter("muT", [P, NB], F32R, isOutput=False)
    sgT_d = nc.declare_dram_parameter("sigmaT", [P, NB], F32R, isOutput=False)
    out_d = nc.declare_dram_parameter("out", [P, NM], F32, isOutput=True)

    with TileContext(nc) as tc, ExitStack() as ctx:
        big = ctx.enter_context(tc.tile_pool(name="big", bufs=1))
        sm = ctx.enter_context(tc.tile_pool(name="sm", bufs=1))
        scr = ctx.enter_context(tc.tile_pool(name="scr", bufs=2))
        pp = ctx.enter_context(tc.tile_pool(name="pp", bufs=2, space="PSUM"))

        # persistent planes, feature-major [128, 4096]
        mu = big.tile([P, NB], F32R)
        sg = big.tile([P, NB], F32R)
        var = big.tile([P, NB], F32R)
        inv = big.tile([P, NB], F32R)
        msq = big.tile([P, NB], F32R)
        m2i = big.tile([P, NB], F32R)
        miv = big.tile([P, NB], F32R)
        muv = big.tile([P, NB], F32R)
        oneh = big.tile([P, RB * NM], F32)  # [128, 2048] stripe one-hots per m

        ones = sm.tile([P, P], F32R)
        ones_f = sm.tile([P, P], F32)
        ioti = sm.tile([P, RB], I32)
        mrow8 = sm.tile([P, 8], F32)
        bias8 = sm.tile([P, 8], F32)
        esum8 = sm.tile([P, 8], F32)
        upos4 = sm.tile([P, NM], F32)

        nc.vector.memset(ones_f, 1.0)
        nc.vector.tensor_copy(out=ones, in_=ones_f)
        # ioti[p, c] = c - p ; onehot_m[p, c] = (c - p == 128*m)
        nc.gpsimd.iota(ioti, pattern=[[1, RB]], base=0, channel_multiplier=-1)
        for m in range(NM):
            nc.vector.tensor_single_scalar(
                out=oneh[:, RB * m:RB * (m + 1)], in_=ioti, scalar=P * m,
                op=ALU.is_equal)

        blk = slice(0, RB)  # this core's block columns (local cols 0..511)
        us_list = []
        c8 = 0
        for h in range(2):
            sl = slice(HALF * h, HALF * (h + 1))
            nc.sync.dma_start(out=mu[:, sl], in_=muT_d[:, sl])
            nc.sync.dma_start(out=sg[:, sl], in_=sgT_d[:, sl])
            nc.vector.tensor_mul(var[:, sl], sg[:, sl], sg[:, sl])
            with nc.allow_low_precision("planes feed the PE which reads fp22"):
                nc.vector.reciprocal(inv[:, sl], var[:, sl])
            nc.vector.tensor_mul(msq[:, sl], mu[:, sl], mu[:, sl])
            nc.vector.scalar_tensor_tensor(
                out=m2i[:, sl], in0=mu[:, sl], scalar=-2.0, in1=inv[:, sl],
                op0=ALU.mult, op1=ALU.mult)
            nc.vector.tensor_mul(miv[:, sl], msq[:, sl], inv[:, sl])
            nc.vector.tensor_copy(out=muv[:, sl], in_=mu[:, sl])

            # chunk 7 (ones, miv) adds c_b = sum_d mu^2*inv to every row of U
            chunks = [(inv, var), (inv, msq), (m2i, muv), (var, inv),
                      (msq, inv), (muv, m2i), (None, miv)]
            for m in range(NM):
                mblk = slice(P * m, P * (m + 1))
                u = pp.tile([P, HALF], F32, name=f"u{h}{m}", tag="ps")
                # Matmul instructions can carry only ONE sync wait in walrus
                # codegen. A psum-slot-reusing group head needs two deps:
                # WAW vs the old matmuls (PE sem) and WAR vs the old tile's
                # DVE read-out. This orphan bf16 ldweights (no PSUM write ->
                # no PE wait) absorbs the DVE dep via the dw token written
                # right after that read-out, leaving the real matmuls with
                # just the PE-completion wait.
                if c8 >= 2:
                    nc.tensor.ldweights(
                        us_list[c8 - 2].bitcast(mybir.dt.bfloat16)[0:1, 0:2])
                for jj in range(4):
                    osl = slice(RB * jj, RB * (jj + 1))
                    gsl = slice(HALF * h + RB * jj, HALF * h + RB * (jj + 1))
                    for ci, (lp, rp) in enumerate(chunks):
                        lhsT = ones if lp is None else lp[:, mblk]
                        nc.tensor.matmul(
                            u[:, osl], lhsT=lhsT, rhs=rp[:, gsl],
                            start=(ci == 0), stop=(ci == len(chunks) - 1))
                if h == 0:
                    # exclude the diagonal (always in local cols 0..511)
                    nc.vector.scalar_tensor_tensor(
                        out=u[:, 0:RB], in0=oneh[:, RB * m:RB * (m + 1)],
                        scalar=1e30, in1=u[:, 0:RB], op0=ALU.mult, op1=ALU.add)
                us = scr.tile([P, HALF], F32, name="us", tag="us", bufs=2)
                us_list.append(us)
                nc.vector.tensor_copy(out=us, in_=u)
                nc.vector.tensor_reduce(mrow8[:, c8:c8 + 1], us, axis=AX.X,
                                        op=ALU.min)
                if h == 1:
                    # positive logits live at local cols 2048+128m+p
                    s512 = scr.tile([P, RB], F32, name="s512", tag="s512",
                                    bufs=1)
                    nc.vector.tensor_mul(s512, us[:, 0:RB],
                                         oneh[:, RB * m:RB * (m + 1)])
                    nc.vector.tensor_reduce(upos4[:, m:m + 1], s512, axis=AX.X,
                                            op=ALU.add)
                nc.vector.tensor_scalar_mul(bias8[:, c8:c8 + 1],
                                            mrow8[:, c8:c8 + 1], SCL)
                e2k = scr.tile([P, HALF], F32, name="e2k", tag="e2k", bufs=1)
                nc.scalar.activation(
                    out=e2k, in_=us, func=AF.Exp, bias=bias8[:, c8:c8 + 1],
                    scale=-SCL, accum_out=esum8[:, c8:c8 + 1])
                c8 += 1

        # tail: per-row LSE = logaddexp(L0, L1); row_loss = LSE + SCL*upos
        logE8 = sm.tile([P, 8], F32)
        nc.scalar.activation(out=logE8, in_=esum8, func=AF.Ln)
        L8 = sm.tile([P, 8], F32)
        nc.vector.scalar_tensor_tensor(out=L8, in0=mrow8, scalar=-SCL,
                                       in1=logE8, op0=ALU.mult, op1=ALU.add)
        L8v = L8.rearrange("p (b a) -> p b a", b=2)  # [:, h, m]
        M4 = sm.tile([P, NM], F32)
        nc.vector.tensor_max(M4, L8v[:, 0, :], L8v[:, 1, :])
        dd = sm.tile([P, 2, NM], F32)
        nc.vector.tensor_sub(dd[:, 0, :], L8v[:, 0, :], M4)
        nc.vector.tensor_sub(dd[:, 1, :], L8v[:, 1, :], M4)
        ee = sm.tile([P, 2, NM], F32)
        nc.scalar.activation(out=ee, in_=dd, func=AF.Exp)
        S4 = sm.tile([P, NM], F32)
        nc.vector.tensor_add(S4, ee[:, 0, :], ee[:, 1, :])
        logS4 = sm.tile([P, NM], F32)
        nc.scalar.activation(out=logS4, in_=S4, func=AF.Ln)
        LSE4 = sm.tile([P, NM], F32)
        nc.vector.tensor_add(LSE4, M4, logS4)
        rl4 = sm.tile([P, NM], F32)
        nc.vector.scalar_tensor_tensor(out=rl4, in0=upos4, scalar=SCL,
                                       in1=LSE4, op0=ALU.mult, op1=ALU.add)
        nc.sync.dma_start(out=out_d[:, :], in_=rl4)

    return nc


_NC_CACHE = None


def _get_nc():
    global _NC_CACHE
    if _NC_CACHE is None:
        nc = _build_nc()
        nc.finalize()  # runs Bacc.compile(): wait legalization for TRN2
        _NC_CACHE = nc
    return _NC_CACHE


def run_sharded(mu_x, sigma_x, mu_p, sigma_p, trace=False):
    mus = np.concatenate([np.asarray(mu_x, np.float32),
                          np.asarray(mu_p, np.float32)], 0)
    sigmas = np.concatenate([np.asarray(sigma_x, np.float32),
                             np.asarray(sigma_p, np.float32)], 0)
    muT = np.ascontiguousarray(mus.T)
    sgT = np.ascontiguousarray(sigmas.T)
    in_maps = [
        {"muT": np.ascontiguousarray(np.roll(muT, -RB * k, axis=1)),
         "sigmaT": np.ascontiguousarray(np.roll(sgT, -RB * k, axis=1))}
        for k in range(NC)
    ]
    kwargs = {}
    if trace:
        kwargs = dict(trace=True, trace_cores=[0])
    br = run_bass_kernel_spmd(_get_nc(), in_maps, core_ids=list(range(NC)),
                              **kwargs)
    total = sum(float(r["out"].astype(np.float64).sum()) for r in br.results)
    n_classes = NB - 1
    to_mult = (n_classes - 1.0 / WEIGHT) / (n_classes - 1)
    to_add = -np.log(np.float32(to_mult))
    loss = np.float32(total / NB - to_add)
    return loss, br


def kernel(z_hat, mu_x, sigma_x, mu_p, sigma_p):
    loss, _ = run_sharded(mu_x, sigma_x, mu_p, sigma_p)
    return np.asarray(loss, np.float32)



# revision 8
# speedup vs baseline: 1.1932x; 1.1932x over previous
"""Trainium2 Bass kernel for nn_ContrastiveDistortion (symmetric pairwise-KL InfoNCE loss).

Math: with IS_SYMMETRIC=True the logdet terms cancel and
  logits_sym[a,b] = D/2 - U[a,b]/4
  U[a,b] = inv_a.(var+msq)_b + (var+msq)_a.inv_b - 2(mu inv)_a.mu_b
           - 2 mu_a.(mu inv)_b + c_a + c_b,   c_x = sum_d msq_x inv_x
c_a is a row constant (drops in log-softmax); c_b is kept as a 5th matmul
chunk with an all-ones lhsT. Each of the 8 cores gets the full [128,4096]
feature-major mu/sigma, column-ROTATED by 512*k so the program is
SPMD-identical: the core's own 512-row block is local columns 0..511
(diagonal masked there) and the positives are local columns 2048..2559.

Per group (h half x m row-chunk) the [128,2048] PSUM tile is consumed in
place (no SBUF copy): DVE masks/extracts/mins, ACT exps with accum_out.
Engine split: ACT squares+exps (one act table: Exp/Ln/Square), DVE
reciprocal+mins+masks+pos, Pool (gpsimd) the remaining elementwise prep.
Warm-up matmuls keep the PE p-state ramped while the first DMA+prep runs.
Per-core partial row-loss sums are reduced on host.
"""

import sys
from contextlib import ExitStack

import numpy as np

sys.path.insert(0, "/opt/trn_rl_repo")

import concourse.bass as bass
import concourse.bacc as bacc_mod
import concourse.mybir as mybir
from concourse.bass_utils import run_bass_kernel_spmd
from concourse.tile import TileContext

F32 = mybir.dt.float32
F32R = mybir.dt.float32r
I32 = mybir.dt.int32
AF = mybir.ActivationFunctionType
ALU = mybir.AluOpType
AX = mybir.AxisListType

P = 128          # partitions / feature dim D
NB = 4096        # N = 2B rows
NC = 8           # cores
RB = NB // NC    # 512 rows per core
NM = RB // P     # 4 m-chunks of 128 rows
HALF = NB // 2   # 2048 columns per softmax half
TEMPERATURE = 0.1
WEIGHT = 5.0
SCL = 1.0 / (4.0 * TEMPERATURE)  # 2.5: l = -SCL*U + const_row
N_WARM = 12      # PE p-state warm-up matmuls during DMA+prep

# column chunking for the DMA->prep pipeline (must sum to NB)
CHUNKS = [512, 512, 1024, 1024, 1024]


def _build_nc():
    nc = bacc_mod.Bacc(None, target_bir_lowering=False, name="contrastive_distortion")
    muT_d = nc.declare_dram_parameter("muT", [P, NB], F32R, isOutput=False)
    sgT_d = nc.declare_dram_parameter("sigmaT", [P, NB], F32R, isOutput=False)
    out_d = nc.declare_dram_parameter("out", [P, NM], F32, isOutput=True)

    with TileContext(nc) as tc, ExitStack() as ctx:
        big = ctx.enter_context(tc.tile_pool(name="big", bufs=1))
        sm = ctx.enter_context(tc.tile_pool(name="sm", bufs=1))
        scr = ctx.enter_context(tc.tile_pool(name="scr", bufs=2))
        pp = ctx.enter_context(tc.tile_pool(name="pp", bufs=2, space="PSUM"))

        # persistent planes, feature-major [128, 4096]
        mu = big.tile([P, NB], F32R)
        sg = big.tile([P, NB], F32R)
        var = big.tile([P, NB], F32R)
        inv = big.tile([P, NB], F32R)
        msq = big.tile([P, NB], F32R)
        vpm = big.tile([P, NB], F32R)   # var + msq
        mui = big.tile([P, NB], F32R)   # mu * inv
        n2mu = big.tile([P, NB], F32R)  # -2 * mu
        miv = big.tile([P, NB], F32R)   # msq * inv  (c_b plane)
        oneh = big.tile([P, RB * NM], F32)  # [128, 2048] stripe one-hots per m

        ones = sm.tile([P, P], F32R)
        ones_f = sm.tile([P, P], F32)
        wrm = sm.tile([P, RB], F32R)   # F32R-rounded rhs for warm-up matmuls
        ioti = sm.tile([P, RB], I32)
        mAB = sm.tile([P, 16], F32)     # per-group half-mins [.., 2*g + part]
        m8 = sm.tile([P, 8], F32)
        bias8 = sm.tile([P, 8], F32)
        esum8 = sm.tile([P, 8], F32)
        upos4 = sm.tile([P, NM], F32)

        nc.vector.memset(ones_f, 1.0)
        nc.vector.tensor_copy(out=ones, in_=ones_f)
        # ioti[p, c] = c - p ; onehot_m[p, c] = (c - p == 128*m)
        nc.gpsimd.iota(ioti, pattern=[[1, RB]], base=0, channel_multiplier=-1)
        for m in range(NM):
            nc.vector.tensor_single_scalar(
                out=oneh[:, RB * m:RB * (m + 1)], in_=ioti, scalar=P * m,
                op=ALU.is_equal)

        # DMA + derived-plane prep, pipelined in column chunks.
        c0 = 0
        for w in CHUNKS:
            sl = slice(c0, c0 + w)
            nc.sync.dma_start(out=sg[:, sl], in_=sgT_d[:, sl])
            nc.sync.dma_start(out=mu[:, sl], in_=muT_d[:, sl])
            nc.scalar.activation(out=var[:, sl], in_=sg[:, sl], func=AF.Square)
            nc.scalar.activation(out=msq[:, sl], in_=mu[:, sl], func=AF.Square)
            nc.scalar.activation(out=n2mu[:, sl], in_=mu[:, sl], func=AF.Copy,
                                 scale=-2.0)
            with nc.allow_low_precision("planes feed the PE which reads fp22"):
                nc.vector.reciprocal(inv[:, sl], var[:, sl])
            if c0 < 1024:
                # latency-critical head chunks: most prep on the fast DVE
                nc.vector.tensor_mul(mui[:, sl], mu[:, sl], inv[:, sl])
                nc.vector.tensor_add(vpm[:, sl], var[:, sl], msq[:, sl])
                nc.gpsimd.tensor_mul(miv[:, sl], msq[:, sl], inv[:, sl])
            else:
                nc.gpsimd.tensor_mul(mui[:, sl], mu[:, sl], inv[:, sl])
                nc.gpsimd.tensor_add(vpm[:, sl], var[:, sl], msq[:, sl])
                nc.gpsimd.tensor_mul(miv[:, sl], msq[:, sl], inv[:, sl])
            c0 += w

        # PE p-state warm-up: garbage matmuls into the first PSUM slot while
        # DMA+prep of chunk 0 is still in flight (start=True overwrites).
        uw = pp.tile([P, HALF], F32, name="uwarm", tag="ps")
        nc.vector.tensor_copy(out=wrm, in_=oneh[:, 0:RB])
        for w in range(N_WARM):
            nc.tensor.matmul(uw[:, 0:RB], lhsT=ones, rhs=wrm,
                             start=True, stop=True)

        blk = slice(0, RB)  # this core's block columns (local cols 0..511)
        e2k_list = []
        c8 = 0
        for h in range(2):
            # chunk 5 (ones, miv) adds c_b = sum_d mu^2*inv to every row of U
            chunks = [(inv, vpm), (vpm, inv), (mui, n2mu), (n2mu, mui),
                      (None, miv)]
            for m in range(NM):
                mblk = slice(P * m, P * (m + 1))
                u = pp.tile([P, HALF], F32, name=f"u{h}{m}", tag="ps")
                # Matmul instructions can carry only ONE sync wait in walrus
                # codegen. A psum-slot-reusing group head needs the WAR deps
                # vs the old tile's DVE min and ACT exp read-outs. Orphan
                # bf16 ldweights (no PSUM write -> no PE wait) absorb those
                # via the tokens written right after the read-outs.
                if c8 >= 2:
                    nc.tensor.ldweights(
                        e2k_list[c8 - 2].bitcast(mybir.dt.bfloat16)[0:1, 0:2])
                    nc.tensor.ldweights(
                        mAB.bitcast(mybir.dt.bfloat16)[0:1, 2 * (c8 - 2):2 * (c8 - 2) + 2])
                for jj in range(4):
                    osl = slice(RB * jj, RB * (jj + 1))
                    gsl = slice(HALF * h + RB * jj, HALF * h + RB * (jj + 1))
                    for ci, (lp, rp) in enumerate(chunks):
                        lhsT = ones if lp is None else lp[:, mblk]
                        nc.tensor.matmul(
                            u[:, osl], lhsT=lhsT, rhs=rp[:, gsl],
                            start=(ci == 0), stop=(ci == len(chunks) - 1))
                    if jj == 0:
                        if h == 0:
                            # exclude the diagonal (always in local cols 0..511)
                            nc.vector.scalar_tensor_tensor(
                                out=u[:, 0:RB], in0=oneh[:, RB * m:RB * (m + 1)],
                                scalar=1e30, in1=u[:, 0:RB],
                                op0=ALU.mult, op1=ALU.add)
                        else:
                            # positive logits live at local cols 2048+128m+p
                            spos = scr.tile([P, RB], F32, name="spos",
                                            tag="spos", bufs=1)
                            nc.vector.tensor_mul(
                                spos, u[:, 0:RB],
                                oneh[:, RB * m:RB * (m + 1)])
                            nc.vector.tensor_reduce(
                                upos4[:, m:m + 1], spos, axis=AX.X,
                                op=ALU.add)
                    if jj == 1:
                        nc.vector.tensor_reduce(
                            mAB[:, 2 * c8:2 * c8 + 1], u[:, 0:2 * RB],
                            axis=AX.X, op=ALU.min)
                # row min of the second half, then combine + scale into bias
                nc.vector.tensor_reduce(
                    mAB[:, 2 * c8 + 1:2 * c8 + 2], u[:, 2 * RB:4 * RB],
                    axis=AX.X, op=ALU.min)
                nc.vector.tensor_tensor(
                    out=m8[:, c8:c8 + 1], in0=mAB[:, 2 * c8:2 * c8 + 1],
                    in1=mAB[:, 2 * c8 + 1:2 * c8 + 2], op=ALU.min)
                nc.vector.tensor_scalar_mul(bias8[:, c8:c8 + 1],
                                            m8[:, c8:c8 + 1], SCL)
                e2k = scr.tile([P, HALF], F32, name="e2k", tag="e2k", bufs=2)
                e2k_list.append(e2k)
                nc.scalar.activation(
                    out=e2k, in_=u, func=AF.Exp, bias=bias8[:, c8:c8 + 1],
                    scale=-SCL, accum_out=esum8[:, c8:c8 + 1])
                c8 += 1

        # tail: per-row LSE = logaddexp over the two halves, one Ln, no
        # table swap (Exp/Ln/Square share the natural_log_exp table).
        m8v = m8.rearrange("p (b a) -> p b a", b=2)  # [:, h, m]
        mmin4 = sm.tile([P, NM], F32)
        nc.vector.tensor_tensor(out=mmin4, in0=m8v[:, 0, :], in1=m8v[:, 1, :],
                                op=ALU.min)
        dd = sm.tile([P, 2, NM], F32)
        nc.vector.tensor_sub(dd[:, 0, :], mmin4, m8v[:, 0, :])
        nc.vector.tensor_sub(dd[:, 1, :], mmin4, m8v[:, 1, :])
        ee = sm.tile([P, 2, NM], F32)
        nc.scalar.activation(out=ee, in_=dd, func=AF.Exp, scale=SCL)
        ws = sm.tile([P, 2, NM], F32)
        nc.vector.tensor_mul(ws, ee, esum8.rearrange("p (b a) -> p b a", b=2))
        s4 = sm.tile([P, NM], F32)
        nc.vector.tensor_add(s4, ws[:, 0, :], ws[:, 1, :])
        l4 = sm.tile([P, NM], F32)
        nc.scalar.activation(out=l4, in_=s4, func=AF.Ln)
        t4 = sm.tile([P, NM], F32)
        nc.vector.tensor_sub(t4, upos4, mmin4)
        rl4 = sm.tile([P, NM], F32)
        nc.vector.scalar_tensor_tensor(out=rl4, in0=t4, scalar=SCL,
                                       in1=l4, op0=ALU.mult, op1=ALU.add)
        nc.sync.dma_start(out=out_d[:, :], in_=rl4)

    return nc


_NC_CACHE = None


def _get_nc():
    global _NC_CACHE
    if _NC_CACHE is None:
        nc = _build_nc()
        nc.finalize()  # runs Bacc.compile(): wait legalization for TRN2
        _NC_CACHE = nc
    return _NC_CACHE


def run_sharded(mu_x, sigma_x, mu_p, sigma_p, trace=False):
    mus = np.concatenate([np.asarray(mu_x, np.float32),
                          np.asarray(mu_p, np.float32)], 0)
    sigmas = np.concatenate([np.asarray(sigma_x, np.float32),
                             np.asarray(sigma_p, np.float32)], 0)
    muT = np.ascontiguousarray(mus.T)
    sgT = np.ascontiguousarray(sigmas.T)
    in_maps = [
        {"muT": np.ascontiguousarray(np.roll(muT, -RB * k, axis=1)),
         "sigmaT": np.ascontiguousarray(np.roll(sgT, -RB * k, axis=1))}
        for k in range(NC)
    ]
    kwargs = {}
    if trace:
        kwargs = dict(trace=True, trace_cores=[0])
    br = run_bass_kernel_spmd(_get_nc(), in_maps, core_ids=list(range(NC)),
                              **kwargs)
    total = sum(float(r["out"].astype(np.float64).sum()) for r in br.results)
    n_classes = NB - 1
    to_mult = (n_classes - 1.0 / WEIGHT) / (n_classes - 1)
    to_add = -np.log(np.float32(to_mult))
    loss = np.float32(total / NB - to_add)
    return loss, br


def kernel(z_hat, mu_x, sigma_x, mu_p, sigma_p):
    loss, _ = run_sharded(mu_x, sigma_x, mu_p, sigma_p)
    return np.asarray(loss, np.float32)


# revision 10
# speedup vs baseline: 1.5696x; 1.3155x over previous
"""Trainium2 Bass kernel for nn_ContrastiveDistortion (symmetric pairwise-KL InfoNCE loss).

Math: with IS_SYMMETRIC=True the logdet terms cancel and
  logits_sym[a,b] = D/2 - U[a,b]/4
  U[a,b] = inv_a.(var+msq)_b + (var+msq)_a.inv_b + (mu inv)_a.(-2mu)_b
           + (-2mu)_a.(mu inv)_b + c_a + c_b,   c_x = sum_d msq_x inv_x
c_a is a row constant (drops in log-softmax); c_b is kept as a 5th matmul
chunk with an all-ones lhsT. Each of the 8 cores gets the full [128,4096]
feature-major mu/sigma, column-ROTATED by 512*k so the program is
SPMD-identical: the core's own 512-row block is local columns 0..511
(diagonal masked there) and the positives are local columns 2048..2559.

Each (h, m) group computes its [128,2048] row-block slab as TWO [128,1024]
PSUM tiles with INDEPENDENT softmax stabilizers (per-sub-tile row min +
exp-sum via ACT accum_out), so tile A's consumers (mask/pos/min/exp)
overlap tile B's matmuls without whole-tile WAR serialization of the PE.
The 16 (min, esum) pairs + 4 positive logits per core are merged on the
host in float64 (exact logaddexp + the weight!=1 positive-logit bump).
Warm-up matmuls hold the PE p-state up while the first DMA+prep runs.
"""

import sys
from contextlib import ExitStack

import numpy as np

sys.path.insert(0, "/opt/trn_rl_repo")

import concourse.bass as bass
import concourse.bacc as bacc_mod
import concourse.mybir as mybir
from concourse.bass_utils import run_bass_kernel_spmd
from concourse.tile import TileContext

F32 = mybir.dt.float32
F32R = mybir.dt.float32r
I32 = mybir.dt.int32
AF = mybir.ActivationFunctionType
ALU = mybir.AluOpType
AX = mybir.AxisListType

P = 128          # partitions / feature dim D
NB = 4096        # N = 2B rows
NC = 8           # cores
RB = NB // NC    # 512 rows per core
NM = RB // P     # 4 m-chunks of 128 rows
HALF = NB // 2   # 2048 columns per softmax half
SUB = 1024       # per-PSUM-tile column count (2 banks)
TEMPERATURE = 0.1
WEIGHT = 5.0
SCL = 1.0 / (4.0 * TEMPERATURE)  # 2.5: l = -SCL*U + const_row
N_WARM = 26      # PE p-state warm-up matmuls (N=128 each) during DMA+prep

# column chunking for the DMA->prep pipeline (must sum to NB)
CHUNKS = [256, 256, 512, 1024, 1024, 1024]


def _build_nc():
    nc = bacc_mod.Bacc(None, target_bir_lowering=False, name="contrastive_distortion")
    muT_d = nc.declare_dram_parameter("muT", [P, NB], F32R, isOutput=False)
    sgT_d = nc.declare_dram_parameter("sigmaT", [P, NB], F32R, isOutput=False)
    # out cols: 0:16 sub-tile mins of U, 16:32 exp sums, 32:36 upos
    out_d = nc.declare_dram_parameter("out", [P, 36], F32, isOutput=True)

    with TileContext(nc) as tc, ExitStack() as ctx:
        big = ctx.enter_context(tc.tile_pool(name="big", bufs=1))
        sm = ctx.enter_context(tc.tile_pool(name="sm", bufs=1))
        scr = ctx.enter_context(tc.tile_pool(name="scr", bufs=4))
        pp = ctx.enter_context(tc.tile_pool(name="pp", bufs=4, space="PSUM"))

        ones = sm.tile([P, P], F32R)
        ones_f = sm.tile([P, P], F32)
        nc.vector.memset(ones_f, 1.0)
        nc.vector.tensor_copy(out=ones, in_=ones_f)

        # PE p-state warm-up: garbage matmuls, deps only on `ones` so the PE
        # is busy from ~0.3us; first real group's start=True overwrites.
        uw = pp.tile([P, SUB], F32, name="uwarm", tag="ps")
        for w in range(N_WARM):
            nc.tensor.matmul(uw[:, 0:P], lhsT=ones, rhs=ones,
                             start=True, stop=True)

        # persistent planes, feature-major [128, 4096]
        mu = big.tile([P, NB], F32R)
        sg = big.tile([P, NB], F32R)
        var = big.tile([P, NB], F32R)
        inv = big.tile([P, NB], F32R)
        msq = big.tile([P, NB], F32R)
        vpm = big.tile([P, NB], F32R)   # var + msq
        mui = big.tile([P, NB], F32R)   # mu * inv
        n2mu = big.tile([P, NB], F32R)  # -2 * mu
        miv = big.tile([P, NB], F32R)   # msq * inv  (c_b plane)

        eye = sm.tile([P, P], F32)      # eye[p, c] = (c == p)
        ioti = sm.tile([P, P], I32)
        mAB = sm.tile([P, 16], F32)     # per-sub-tile row mins of U
        bias16 = sm.tile([P, 16], F32)
        esum16 = sm.tile([P, 16], F32)
        upos4 = sm.tile([P, NM], F32)
        outt = sm.tile([P, 36], F32)

        nc.gpsimd.iota(ioti, pattern=[[1, P]], base=0, channel_multiplier=-1)
        nc.vector.tensor_single_scalar(out=eye, in_=ioti, scalar=0,
                                       op=ALU.is_equal)

        # DMA + derived-plane prep, pipelined in column chunks.
        c0 = 0
        for w in CHUNKS:
            sl = slice(c0, c0 + w)
            nc.sync.dma_start(out=sg[:, sl], in_=sgT_d[:, sl])
            nc.sync.dma_start(out=mu[:, sl], in_=muT_d[:, sl])
            nc.scalar.activation(out=var[:, sl], in_=sg[:, sl], func=AF.Square)
            nc.scalar.activation(out=msq[:, sl], in_=mu[:, sl], func=AF.Square)
            with nc.allow_low_precision("planes feed the PE which reads fp22"):
                nc.vector.reciprocal(inv[:, sl], var[:, sl])
            if c0 < 1024:
                # latency-critical head chunks: fast engines
                nc.scalar.activation(out=n2mu[:, sl], in_=mu[:, sl],
                                     func=AF.Copy, scale=-2.0)
                nc.vector.tensor_mul(mui[:, sl], mu[:, sl], inv[:, sl])
                nc.vector.tensor_add(vpm[:, sl], var[:, sl], msq[:, sl])
                nc.gpsimd.tensor_mul(miv[:, sl], msq[:, sl], inv[:, sl])
            else:
                nc.gpsimd.tensor_scalar_mul(n2mu[:, sl], mu[:, sl], -2.0)
                nc.gpsimd.tensor_mul(mui[:, sl], mu[:, sl], inv[:, sl])
                nc.gpsimd.tensor_add(vpm[:, sl], var[:, sl], msq[:, sl])
                nc.gpsimd.tensor_mul(miv[:, sl], msq[:, sl], inv[:, sl])
            c0 += w

        chunks = [(inv, vpm), (vpm, inv), (mui, n2mu), (n2mu, mui), (None, miv)]
        e_list = []
        g16 = 0
        for h in range(2):
            for m in range(NM):
                mblk = slice(P * m, P * (m + 1))
                dblk = slice(P * m, P * (m + 1))  # diag/pos block inside jj0
                ts_ = []
                for half2 in range(2):  # tile A (jj0-1) then tile B (jj2-3)
                    t = pp.tile([P, SUB], F32, name=f"u{h}{m}{half2}", tag="ps")
                    ts_.append(t)
                    # single-wait legalization: orphan ldweights absorb the
                    # WAR deps vs the slot's previous readers (DVE min write
                    # token + ACT exp output token).
                    if g16 >= 4:
                        nc.tensor.ldweights(
                            mAB.bitcast(mybir.dt.bfloat16)[
                                0:1, 2 * (g16 - 4):2 * (g16 - 4) + 2])
                        nc.tensor.ldweights(
                            e_list[g16 - 4].bitcast(mybir.dt.bfloat16)[0:1, 0:2])
                    for jj in range(2):
                        osl = slice(RB * jj, RB * (jj + 1))
                        gsl = slice(HALF * h + SUB * half2 + RB * jj,
                                    HALF * h + SUB * half2 + RB * (jj + 1))
                        for ci, (lp, rp) in enumerate(chunks):
                            lhsT = ones if lp is None else lp[:, mblk]
                            nc.tensor.matmul(
                                t[:, osl], lhsT=lhsT, rhs=rp[:, gsl],
                                start=(ci == 0), stop=(ci == len(chunks) - 1))
                    if half2 == 0:
                        if h == 0:
                            # mask the diagonal (block at jj0 cols 128m..)
                            nc.vector.scalar_tensor_tensor(
                                out=t[:, dblk], in0=eye, scalar=1e30,
                                in1=t[:, dblk], op0=ALU.mult, op1=ALU.add)
                        else:
                            # positive logits at local cols 2048+128m+p
                            spos = scr.tile([P, P], F32, name="spos",
                                            tag="spos", bufs=2)
                            nc.vector.tensor_mul(spos, t[:, dblk], eye)
                            nc.vector.tensor_reduce(
                                upos4[:, m:m + 1], spos, axis=AX.X, op=ALU.add)
                    # per-sub-tile row min -> bias -> exp with accum_out
                    nc.vector.tensor_reduce(mAB[:, g16:g16 + 1], t,
                                            axis=AX.X, op=ALU.min)
                    nc.vector.tensor_scalar_mul(bias16[:, g16:g16 + 1],
                                                mAB[:, g16:g16 + 1], SCL)
                    e2k = scr.tile([P, SUB], F32, name="e2k", tag="e2k",
                                   bufs=4)
                    e_list.append(e2k)
                    nc.scalar.activation(
                        out=e2k, in_=t, func=AF.Exp,
                        bias=bias16[:, g16:g16 + 1], scale=-SCL,
                        accum_out=esum16[:, g16:g16 + 1])
                    g16 += 1

        nc.vector.tensor_copy(out=outt[:, 0:16], in_=mAB)
        nc.vector.tensor_copy(out=outt[:, 16:32], in_=esum16)
        nc.vector.tensor_copy(out=outt[:, 32:36], in_=upos4)
        nc.sync.dma_start(out=out_d[:, :], in_=outt)

    return nc


_NC_CACHE = None


def _get_nc():
    global _NC_CACHE
    if _NC_CACHE is None:
        nc = _build_nc()
        nc.finalize()  # runs Bacc.compile(): wait legalization for TRN2
        _NC_CACHE = nc
    return _NC_CACHE


def run_sharded(mu_x, sigma_x, mu_p, sigma_p, trace=False):
    mus = np.concatenate([np.asarray(mu_x, np.float32),
                          np.asarray(mu_p, np.float32)], 0)
    sigmas = np.concatenate([np.asarray(sigma_x, np.float32),
                             np.asarray(sigma_p, np.float32)], 0)
    muT = np.ascontiguousarray(mus.T)
    sgT = np.ascontiguousarray(sigmas.T)
    in_maps = [
        {"muT": np.ascontiguousarray(np.roll(muT, -RB * k, axis=1)),
         "sigmaT": np.ascontiguousarray(np.roll(sgT, -RB * k, axis=1))}
        for k in range(NC)
    ]
    kwargs = {}
    if trace:
        kwargs = dict(trace=True, trace_cores=[0])
    br = run_bass_kernel_spmd(_get_nc(), in_maps, core_ids=list(range(NC)),
                              **kwargs)

    # Host-side float64 merge of per-sub-tile softmax stats.
    # Sub-tile g16 = ((h*NM + m)*2 + half2) holds rows (m, p): min_u and
    # esum = sum_cols exp(-SCL*(u - min_u)); row max-logit M_g = -SCL*min_u.
    n_classes = NB - 1
    to_mult = (n_classes - 1.0 / WEIGHT) / (n_classes - 1)
    to_add = -np.log(np.float64(to_mult))
    total = 0.0
    for r in br.results:
        o = r["out"].astype(np.float64)          # [128, 36]
        mins = o[:, 0:16].reshape(P, 2, NM, 2)   # [p, h, m, half2]
        esum = o[:, 16:32].reshape(P, 2, NM, 2)
        upos = o[:, 32:36]                       # [p, m]
        # row (m, p) merges its 4 sub-tiles (h x half2): LSE with the exact
        # weight!=1 bump on the positive logit.
        for m in range(NM):
            M = -SCL * mins[:, :, m, :]          # [p, 2, 2] sub-tile maxima
            es = esum[:, :, m, :]
            Mstar = M.max(axis=(1, 2))           # [p]
            s = (es * np.exp(M - Mstar[:, None, None])).sum(axis=(1, 2))
            lpos = -SCL * upos[:, m]             # [p]
            s += (np.exp(to_add) - 1.0) * np.exp(lpos - Mstar)
            lse = Mstar + np.log(s)
            total += (lse - (lpos + to_add)).sum()
    loss = np.float32(total / NB)
    return loss, br


def kernel(z_hat, mu_x, sigma_x, mu_p, sigma_p):
    loss, _ = run_sharded(mu_x, sigma_x, mu_p, sigma_p)
    return np.asarray(loss, np.float32)


# revision 11
# speedup vs baseline: 1.7354x; 1.1056x over previous
"""Trainium2 Bass kernel for nn_ContrastiveDistortion (symmetric pairwise-KL InfoNCE loss).

Math: with IS_SYMMETRIC=True the logdet terms cancel and
  logits_sym[a,b] = D/2 - U[a,b]/4
  U[a,b] = inv_a.(var+msq)_b + (var+msq)_a.inv_b + (mu inv)_a.(-2mu)_b
           + (-2mu)_a.(mu inv)_b + c_a + c_b,   c_x = sum_d msq_x inv_x
c_a is a row constant (drops in log-softmax); c_b is kept as a 5th matmul
chunk with an all-ones lhsT. Each of the 8 cores gets the full [128,4096]
feature-major mu/sigma, column-ROTATED by 512*k so the program is
SPMD-identical: the core's own 512-row block is local columns 0..511
(diagonal masked there) and the positives are local columns 2048..2559.

The [512, 4096] row-block slab is computed as 20 PSUM tiles, ordered in
four column phases (h0A 0:1024 | h0B 1024:2048 | h1A 2048:3072 | h1B
3072:4096) of the four 128-row m-chunks each, so each phase needs only
1024 freshly-prepped columns and covers the next chunk's prep with ~8.5us
of matmuls. Every tile carries an INDEPENDENT softmax stabilizer (row min
+ exp-sum via ACT accum_out) consumed in place from PSUM - no SBUF copy,
no whole-slab WAR serialization. Phase 3 uses 512-wide half tiles to
shorten the final drain chain. The per-tile (min, esum) pairs + positive
logits are merged on the host in float64 (exact logaddexp + the weight!=1
positive-logit bump). Warm-up matmuls hold the PE p-state up while the
first DMA+prep runs.
"""

import sys
from contextlib import ExitStack

import numpy as np

sys.path.insert(0, "/opt/trn_rl_repo")

import concourse.bass as bass
import concourse.bacc as bacc_mod
import concourse.mybir as mybir
from concourse.bass_utils import run_bass_kernel_spmd
from concourse.tile import TileContext

F32 = mybir.dt.float32
F32R = mybir.dt.float32r
I32 = mybir.dt.int32
AF = mybir.ActivationFunctionType
ALU = mybir.AluOpType
AX = mybir.AxisListType

P = 128          # partitions / feature dim D
NB = 4096        # N = 2B rows
NC = 8           # cores
RB = NB // NC    # 512 rows per core
NM = RB // P     # 4 m-chunks of 128 rows
HALF = NB // 2   # 2048 columns per softmax half
SUB = 1024       # per-PSUM-tile column count in phases 0-2 (2 banks)
NST = 20         # stat slots: 12 full tiles + 8 half tiles
TEMPERATURE = 0.1
WEIGHT = 5.0
SCL = 1.0 / (4.0 * TEMPERATURE)  # 2.5: l = -SCL*U + const_row
N_WARM = 30      # PE p-state warm-up matmuls (N=128 each) during DMA+prep

# column chunking for the DMA->prep pipeline (must sum to NB)
CHUNKS = [256, 256, 512, 1024, 1024, 1024]


def _build_nc():
    nc = bacc_mod.Bacc(None, target_bir_lowering=False, name="contrastive_distortion")
    muT_d = nc.declare_dram_parameter("muT", [P, NB], F32R, isOutput=False)
    sgT_d = nc.declare_dram_parameter("sigmaT", [P, NB], F32R, isOutput=False)
    # out cols: 0:20 per-tile row mins of U, 20:40 exp sums, 40:44 upos
    out_d = nc.declare_dram_parameter("out", [P, 44], F32, isOutput=True)

    with TileContext(nc) as tc, ExitStack() as ctx:
        big = ctx.enter_context(tc.tile_pool(name="big", bufs=1))
        sm = ctx.enter_context(tc.tile_pool(name="sm", bufs=1))
        scr = ctx.enter_context(tc.tile_pool(name="scr", bufs=4))
        pp = ctx.enter_context(tc.tile_pool(name="pp", bufs=4, space="PSUM"))

        ones = sm.tile([P, P], F32R)
        ones_f = sm.tile([P, P], F32)
        nc.vector.memset(ones_f, 1.0)
        nc.vector.tensor_copy(out=ones, in_=ones_f)

        # PE p-state warm-up: garbage matmuls, deps only on `ones` so the PE
        # is busy from ~0.3us; the first real group's start=True overwrites.
        uw = pp.tile([P, SUB], F32, name="uwarm", tag="ps")
        for w in range(N_WARM):
            nc.tensor.matmul(uw[:, 0:P], lhsT=ones, rhs=ones,
                             start=True, stop=True)

        # persistent planes, feature-major [128, 4096]
        mu = big.tile([P, NB], F32R)
        sg = big.tile([P, NB], F32R)
        var = big.tile([P, NB], F32R)
        inv = big.tile([P, NB], F32R)
        msq = big.tile([P, NB], F32R)
        vpm = big.tile([P, NB], F32R)   # var + msq
        mui = big.tile([P, NB], F32R)   # mu * inv
        n2mu = big.tile([P, NB], F32R)  # -2 * mu
        miv = big.tile([P, NB], F32R)   # msq * inv  (c_b plane)

        eye = sm.tile([P, P], F32)      # eye[p, c] = (c == p)
        ioti = sm.tile([P, P], I32)
        outt = sm.tile([P, 44], F32)
        mins = outt[:, 0:NST]
        esums = outt[:, NST:2 * NST]
        upos4 = outt[:, 2 * NST:2 * NST + NM]
        bias20 = sm.tile([P, NST], F32)

        nc.gpsimd.iota(ioti, pattern=[[1, P]], base=0, channel_multiplier=-1)
        nc.vector.tensor_single_scalar(out=eye, in_=ioti, scalar=0,
                                       op=ALU.is_equal)

        # DMA + derived-plane prep, pipelined in column chunks. Cols 0:1024
        # (phase-0 critical) on DVE/ACT; the rest on the otherwise-idle Pool.
        c0 = 0
        for w in CHUNKS:
            sl = slice(c0, c0 + w)
            nc.sync.dma_start(out=sg[:, sl], in_=sgT_d[:, sl])
            nc.sync.dma_start(out=mu[:, sl], in_=muT_d[:, sl])
            nc.scalar.activation(out=var[:, sl], in_=sg[:, sl], func=AF.Square)
            nc.scalar.activation(out=msq[:, sl], in_=mu[:, sl], func=AF.Square)
            with nc.allow_low_precision("planes feed the PE which reads fp22"):
                nc.vector.reciprocal(inv[:, sl], var[:, sl])
            if c0 < 1024:
                nc.scalar.activation(out=n2mu[:, sl], in_=mu[:, sl],
                                     func=AF.Copy, scale=-2.0)
                nc.vector.tensor_mul(mui[:, sl], mu[:, sl], inv[:, sl])
                nc.vector.tensor_add(vpm[:, sl], var[:, sl], msq[:, sl])
                nc.gpsimd.tensor_mul(miv[:, sl], msq[:, sl], inv[:, sl])
            else:
                nc.gpsimd.tensor_scalar_mul(n2mu[:, sl], mu[:, sl], -2.0)
                nc.gpsimd.tensor_mul(mui[:, sl], mu[:, sl], inv[:, sl])
                nc.gpsimd.tensor_add(vpm[:, sl], var[:, sl], msq[:, sl])
                nc.gpsimd.tensor_mul(miv[:, sl], msq[:, sl], inv[:, sl])
            c0 += w

        chunks = [(inv, vpm), (vpm, inv), (mui, n2mu), (n2mu, mui), (None, miv)]
        e_list = []   # (stats_slot, e_tile) per PSUM tile, for the orphans
        g20 = 0
        # phases: (column offset, tile width); phase 3 split into halves
        tiles = []
        for phase in range(3):
            for m in range(NM):
                tiles.append((phase * SUB, SUB, m, phase))
        for m in range(NM):
            for hh in range(2):
                tiles.append((3 * SUB + 512 * hh, 512, m, 3))

        for ti, (coff, width, m, phase) in enumerate(tiles):
            mblk = slice(P * m, P * (m + 1))
            t = pp.tile([P, width], F32, name=f"t{ti}", tag="ps")
            # Matmuls carry ONE wait in walrus codegen; a slot-reusing tile
            # head has WAR deps vs the old tile's DVE min and ACT exp reads.
            # Orphan bf16 ldweights (no PSUM write) absorb those via the
            # tokens written right after the read-outs.
            if ti >= 4:
                os_, _ = e_list[ti - 4]
                nc.tensor.ldweights(
                    bias20.bitcast(mybir.dt.bfloat16)[0:1, 2 * os_:2 * os_ + 2])
                nc.tensor.ldweights(
                    e_list[ti - 4][1].bitcast(mybir.dt.bfloat16)[0:1, 0:2])
            for jj in range(width // RB):
                osl = slice(RB * jj, RB * (jj + 1))
                gsl = slice(coff + RB * jj, coff + RB * (jj + 1))
                for ci, (lp, rp) in enumerate(chunks):
                    lhsT = ones if lp is None else lp[:, mblk]
                    nc.tensor.matmul(
                        t[:, osl], lhsT=lhsT, rhs=rp[:, gsl],
                        start=(ci == 0), stop=(ci == len(chunks) - 1))
            if phase == 0:
                # mask the diagonal (block at cols 128m.. of phase 0)
                dblk = slice(P * m, P * (m + 1))
                nc.vector.scalar_tensor_tensor(
                    out=t[:, dblk], in0=eye, scalar=1e30,
                    in1=t[:, dblk], op0=ALU.mult, op1=ALU.add)
            if phase == 2:
                # positive logits at local cols 2048+128m+p
                dblk = slice(P * m, P * (m + 1))
                spos = scr.tile([P, P], F32, name="spos", tag="spos", bufs=2)
                nc.vector.tensor_mul(spos, t[:, dblk], eye)
                nc.vector.tensor_reduce(
                    upos4[:, m:m + 1], spos, axis=AX.X, op=ALU.add)
            # per-tile row min -> bias -> exp with accum_out, all from PSUM
            nc.vector.tensor_reduce(mins[:, g20:g20 + 1], t,
                                    axis=AX.X, op=ALU.min)
            nc.vector.tensor_scalar_mul(bias20[:, g20:g20 + 1],
                                        mins[:, g20:g20 + 1], SCL)
            e2k = scr.tile([P, width], F32, name="e2k", tag="e2k", bufs=4)
            e_list.append((g20, e2k))
            nc.scalar.activation(
                out=e2k, in_=t, func=AF.Exp,
                bias=bias20[:, g20:g20 + 1], scale=-SCL,
                accum_out=esums[:, g20:g20 + 1])
            g20 += 1

        nc.sync.dma_start(out=out_d[:, :], in_=outt)

    return nc


_NC_CACHE = None


def _get_nc():
    global _NC_CACHE
    if _NC_CACHE is None:
        nc = _build_nc()
        nc.finalize()  # runs Bacc.compile(): wait legalization for TRN2
        _NC_CACHE = nc
    return _NC_CACHE


def run_sharded(mu_x, sigma_x, mu_p, sigma_p, trace=False):
    mus = np.concatenate([np.asarray(mu_x, np.float32),
                          np.asarray(mu_p, np.float32)], 0)
    sigmas = np.concatenate([np.asarray(sigma_x, np.float32),
                             np.asarray(sigma_p, np.float32)], 0)
    muT = np.ascontiguousarray(mus.T)
    sgT = np.ascontiguousarray(sigmas.T)
    in_maps = [
        {"muT": np.ascontiguousarray(np.roll(muT, -RB * k, axis=1)),
         "sigmaT": np.ascontiguousarray(np.roll(sgT, -RB * k, axis=1))}
        for k in range(NC)
    ]
    kwargs = {}
    if trace:
        kwargs = dict(trace=True, trace_cores=[0])
    br = run_bass_kernel_spmd(_get_nc(), in_maps, core_ids=list(range(NC)),
                              **kwargs)

    # Host-side float64 merge of per-tile softmax stats. Stat slot layout:
    # slots 0:4 = phase h0A (cols 0:1024) for m=0..3, 4:8 = h0B, 8:12 = h1A,
    # 12:20 = h1B split in 512-halves: slot 12+2m+hh. Row (m, p) merges its
    # 5 slots; max logit per slot = -SCL*min_u.
    n_classes = NB - 1
    to_mult = (n_classes - 1.0 / WEIGHT) / (n_classes - 1)
    to_add = -np.log(np.float64(to_mult))
    total = 0.0
    for r in br.results:
        o = r["out"].astype(np.float64)          # [128, 44]
        mins = o[:, 0:NST]
        esum = o[:, NST:2 * NST]
        upos = o[:, 2 * NST:2 * NST + NM]        # [p, m]
        for m in range(NM):
            slots = [m, 4 + m, 8 + m, 12 + 2 * m, 13 + 2 * m]
            M = -SCL * mins[:, slots]            # [p, 5] per-slot max logit
            es = esum[:, slots]
            Mstar = M.max(axis=1)                # [p]
            s = (es * np.exp(M - Mstar[:, None])).sum(axis=1)
            lpos = -SCL * upos[:, m]             # [p]
            # exact weight!=1 bump of the positive logit inside the LSE
            s += (np.exp(to_add) - 1.0) * np.exp(lpos - Mstar)
            lse = Mstar + np.log(s)
            total += (lse - (lpos + to_add)).sum()
    loss = np.float32(total / NB)
    return loss, br


def kernel(z_hat, mu_x, sigma_x, mu_p, sigma_p):
    loss, _ = run_sharded(mu_x, sigma_x, mu_p, sigma_p)
    return np.asarray(loss, np.float32)
